# revision 6
# baseline (speedup 1.0000x reference)
"""MoCo loss (InfoNCE over a 65536-entry queue + proto-NCE over 50000
k-means centroids) on 8 Trainium2 NeuronCores.

fp8e4m3 operands with DoubleRowSwInterleave matmuls (2 contraction
subtiles per instruction; the stationary operand is pre-interleaved on
the host -- A/B k-layer pairs per column, columns reversed -- so the
weight load reads contiguously).  Tables are scaled by 16 per operand
(dots come out x256) and quantized to e4m3 on the host.

Per core (tables sharded by row, Z_q replicated):

  part 2 (centroid shard, zq stationary / centroids moving):
    - s2 = Z_q @ c_shard.T  (PE, fp32 acc), cast fp8 (DVE/ACT), export
      (argmax + exclusion gather on the host)
  part 1 (queue shard, queue stationary / zq moving):
    - s1 = q_shard @ Z_q.T                 (PE)
    - exp(s1/T) in fp8  (no shift; values in [e^-4, e^4])   (ACT)
    - per-queue-row max of the exp tiles -> rm export       (DVE)
    - ones DoubleRow matmul partition sum -> per-batch partial
      sum-of-exp                                            (PE)

The kernel is HBM-stream-bound: ~7.3 MiB of table reads per core plus
exports.  Exports are kept tiny (s2 fp8 1.5 MiB, rm 32 KiB, fin 1 KiB)
and inputs stream on both HWDGE rings (sync + scalar) while s2 exports
ride the gpsimd (SWDGE) ring, so the 16 SDMA engines stay saturated.

The host combines per-core partials (logsumexp merge, global argmax,
exclusion gather + 513-wide softmax).  Both outputs stay exact despite
fp8 noise: the host computes s0 = queue @ Z_q[0] itself and re-checks
every queue row within MARGIN of the device rowmax -- and every
centroid column within MARGIN2 of its row max -- with full-precision
dot products (fp8 dot error is < 0.01).
"""

import os
import numpy as np
import ml_dtypes

B, C = 256, 512
QUEUE, NCL, NNEG = 65536, 50000, 512
INFO_TEMP = 0.07
PROTO_FACTOR = 0.5
NCORES = 8
QSH = QUEUE // NCORES          # 8192 queue rows per core
CSH = NCL // NCORES            # 6250 centroid rows per core
CSH_PAD = 6272                 # 14 * 448
CCH = 14                       # s2 matmul chunks
CW = CSH_PAD // CCH            # 448
KSUB = C // 128                # 4 contraction subtiles
KPAIR = KSUB // 2              # 2 DoubleRow pairs
NBT = 16                       # part-1 big tiles (512 queue rows each)
QCHUNK = 4                     # qT DMA chunks (8 KiB per-partition lines)
JW = QSH // QCHUNK             # 2048
JSUB = JW // 128               # 16 queue subtiles per chunk
FP8_SCALE = 16.0               # per-operand scale; dots come out x256
DOT_SCALE = FP8_SCALE * FP8_SCALE
MARGIN = 0.05                  # host re-check threshold (unscaled units)
MARGIN2 = 0.045                # part-2 argmax re-check threshold

ACT_CAST_CH = (0, 1, 2, 3)     # s2 chunks cast on ACT (idle early); rest DVE
# rm reduce groups: big groups early, small at the tail so the last
# reduce (and the rm export behind it) has a short critical chain
RM_GROUPS = ((0, 4), (4, 8), (8, 12), (12, 14), (14, 15), (15, 16))

_CACHE = {}

# exec time of the last device run (ns), populated when tracing is on
last_exec_time_ns = None


def _build():
    import concourse.bass as bass
    import concourse.tile as tile
    from concourse import bacc, mybir

    dt = mybir.dt
    DRI = mybir.MatmulPerfMode.DoubleRowSwInterleave
    nc = bacc.Bacc(
        "TRN2", target_bir_lowering=False, debug=False, num_devices=NCORES
    )

    # ---- DRAM I/O (all partition-major so every DMA is [128, N] flat) ----
    # zq packs both Z_q layouts in one transfer: [:, 0:4, :] = zqT (moving
    # operand of part 1, k-pair slices), [:, 4:8, :] = zqTi (part-2
    # stationary, host-interleaved per (kpair, bt))
    zq_d = nc.dram_tensor("zq", [128, 8, 256], dt.float8e4, kind="ExternalInput").ap()
    # qT: part-1 stationary, host-interleaved per (jsub, kpair)
    qT_d = nc.dram_tensor(
        "qT", [QCHUNK, 128, JSUB, KPAIR, 256], dt.float8e4, kind="ExternalInput"
    ).ap()
    # cT split 3x4 + 1x2 matmul chunks for big per-partition lines
    cTa_d = nc.dram_tensor(
        "cTa", [3, 128, KSUB, 4 * CW], dt.float8e4, kind="ExternalInput"
    ).ap()
    cTb_d = nc.dram_tensor(
        "cTb", [128, KSUB, 2 * CW], dt.float8e4, kind="ExternalInput"
    ).ap()

    # fin: per-batch partial sum-of-exp (all 128 psum partitions carry the
    # same value -- export one row)
    fin_d = nc.dram_tensor("fin", [1, 256], dt.float32, kind="ExternalOutput").ap()
    # rm: per-queue-row max over the batch of the fp8 exp tiles (fp32 holds
    # the fp8 value exactly, so the host margin test matches a host-side max)
    rm_d = nc.dram_tensor("rm", [128, NBT, 4], dt.float32, kind="ExternalOutput").ap()
    s2_d = nc.dram_tensor(
        "s2", [128, CCH, 2, CW], dt.float8e4, kind="ExternalOutput"
    ).ap()

    with tile.TileContext(nc) as tc:
        with (
            tc.tile_pool(name="const", bufs=1) as cpool,
            tc.tile_pool(name="ps1", bufs=2, space="PSUM") as ps1,
            tc.tile_pool(name="psum1s", bufs=1, space="PSUM") as ps1s,
            tc.tile_pool(name="ps2", bufs=3, space="PSUM") as ps2,
        ):
            # ---- resident SBUF tensors ----
            zq_sb = cpool.tile([128, 8, 256], dt.float8e4)
            cT_sb = [
                cpool.tile(
                    [128, KSUB, 4 * CW], dt.float8e4, name=f"cTa{ch}", tag=f"cT{ch}"
                )
                for ch in range(3)
            ]
            cTb_sb = cpool.tile([128, KSUB, 2 * CW], dt.float8e4)
            qt_sb = [
                cpool.tile(
                    [128, JSUB, KPAIR, 256], dt.float8e4, name=f"qt{h}", tag=f"qt{h}"
                )
                for h in range(QCHUNK)
            ]

            # Input DMAs on the two HWDGE rings (sync + scalar), issue order
            # = arrival order.  Small zq/cTb first so the PE starts early;
            # the last chunks are ones with short downstream chains.
            #   sync:   zq, qt0, cTa1, qt2   (+ rm, fin exports at the end)
            #   scalar: cTb, cTa0, qt1, qt3, cTa2
            # s2 exports ride the gpsimd (SWDGE) ring.
            nc.sync.dma_start(zq_sb[:], zq_d[:])
            nc.scalar.dma_start(cTb_sb[:], cTb_d[:])
            nc.scalar.dma_start(cT_sb[0][:], cTa_d[0])
            nc.sync.dma_start(qt_sb[0][:], qT_d[0])
            nc.scalar.dma_start(qt_sb[1][:], qT_d[1])
            nc.sync.dma_start(cT_sb[1][:], cTa_d[1])
            nc.sync.dma_start(qt_sb[2][:], qT_d[2])
            nc.scalar.dma_start(qt_sb[3][:], qT_d[3])
            nc.scalar.dma_start(cT_sb[2][:], cTa_d[2])

            # interleave/column-reversal of all-ones is all-ones
            ones_sb = cpool.tile([128, 256], dt.float8e4)
            nc.vector.memset(ones_sb[:], 1.0)

            # ---- part 2: centroid shard (argmax happens on the host) ----
            # fp8 export: the host re-checks near-max columns exactly, and
            # pl_neg noise averages out in the 513-wide softmax
            s2_sb = cpool.tile([128, CCH, 2, CW], dt.float8e4)

            for ch in range(CCH):
                if ch < 2:
                    cmov, w = cTb_sb, ch
                else:
                    dch, w = divmod(ch - 2, 4)
                    cmov = cT_sb[dch]
                for bt in range(2):
                    s2_ps = ps2.tile([128, CW], dt.float32, tag="s2")
                    for kp in range(KPAIR):
                        nc.tensor.matmul(
                            s2_ps[:],
                            zq_sb[:, 4 + 2 * kp + bt, :],
                            cmov[:, 2 * kp : 2 * kp + 2, w * CW : (w + 1) * CW],
                            start=(kp == 0),
                            stop=(kp == KPAIR - 1),
                            perf_mode=DRI,
                        )
                    # psum -> sbuf fp8 cast; early chunks on ACT (idle until
                    # the first part-1 exp), the rest on DVE
                    if ch in ACT_CAST_CH:
                        nc.scalar.copy(s2_sb[:, ch, bt, :], s2_ps[:])
                    else:
                        nc.vector.tensor_copy(s2_sb[:, ch, bt, :], s2_ps[:])
                if ch == 7:
                    # first 8 chunk-columns done for both bt: export early so
                    # the write overlaps the rest of the kernel
                    nc.gpsimd.dma_start(
                        s2_d[:, 0:8].rearrange("p c b w -> p (c b w)"),
                        s2_sb[:, 0:8].rearrange("p c b w -> p (c b w)"),
                    )
            nc.gpsimd.dma_start(
                s2_d[:, 8:CCH].rearrange("p c b w -> p (c b w)"),
                s2_sb[:, 8:CCH].rearrange("p c b w -> p (c b w)"),
            )

            # ---- part 1: queue shard, 16 big tiles of 512 rows ----
            fin_sb = cpool.tile([128, 256], dt.float32)     # p1sum
            rm_sb = cpool.tile([128, NBT, 4], dt.float32)   # per-row max
            p1s_ps = ps1s.tile([128, B], dt.float32)        # sum-of-exp accum

            exp_all = cpool.tile([128, NBT, 4, B], dt.float8e4)
            exp_tiles = [exp_all[:, t] for t in range(NBT)]
            rm_after = {b - 1: (a, b) for a, b in RM_GROUPS}
            for t in range(NBT):
                s1_ps = ps1.tile([128, 4, B], dt.float32, tag="s1")
                for q in range(4):
                    jt = t * 4 + q
                    h, jl = divmod(jt, JSUB)
                    for kp in range(KPAIR):
                        nc.tensor.matmul(
                            s1_ps[:, q, :],
                            qt_sb[h][:, jl, kp, :],
                            zq_sb[:, 2 * kp : 2 * kp + 2, :],
                            start=(kp == 0),
                            stop=(kp == KPAIR - 1),
                            perf_mode=DRI,
                        )
                exp_t = exp_tiles[t]
                nc.scalar.activation(
                    exp_t[:],
                    s1_ps[:],
                    mybir.ActivationFunctionType.Exp,
                    scale=1.0 / (DOT_SCALE * INFO_TEMP),
                )
                if t > 1:
                    # two tiles behind: ACT's exp has a full tile of slack
                    for g in range(2):
                        nc.tensor.matmul(
                            p1s_ps[:],
                            ones_sb[:],
                            exp_tiles[t - 2][:, 2 * g : 2 * g + 2, :],
                            start=(t == 2 and g == 0),
                            stop=False,
                            perf_mode=DRI,
                        )
                if t in rm_after:
                    # per-queue-row max over the batch (free axis) of the
                    # fp8 exp tiles just completed
                    a, b = rm_after[t]
                    nc.vector.tensor_reduce(
                        rm_sb[:, a:b],
                        exp_all[:, a:b],
                        axis=mybir.AxisListType.X,
                        op=mybir.AluOpType.max,
                    )
            for t in (NBT - 2, NBT - 1):
                for g in range(2):
                    nc.tensor.matmul(
                        p1s_ps[:],
                        ones_sb[:],
                        exp_tiles[t][:, 2 * g : 2 * g + 2, :],
                        start=False,
                        stop=(t == NBT - 1 and g == 1),
                        perf_mode=DRI,
                    )

            nc.sync.dma_start(rm_d[:], rm_sb[:])
            nc.scalar.copy(fin_sb[:], p1s_ps[:])
            nc.sync.dma_start(fin_d[:], fin_sb[0:1, :])

    nc.compile()
    return nc


def _get_nc():
    if "nc" not in _CACHE:
        _CACHE["nc"] = _build()
    return _CACHE["nc"]


def _to_fp8(x):
    return (x * FP8_SCALE).astype(ml_dtypes.float8_e4m3fn)


def _interleave(A, B):
    """SwInterleave weight layout: mem[p, 2*jj+i] = layer_i[p, 127-jj].
    A, B: [..., 128, 128] (partition, column)."""
    return np.stack([A[..., ::-1], B[..., ::-1]], axis=-1).reshape(
        *A.shape[:-1], 256
    )


def _prep_inputs(Z_q, queue, centroids):
    """Host-side shard prep: x16 scale + e4m3 quantization + transpose to
    [C, rows], then partition-major chunk layouts so each DMA is a flat
    [128, N].  Stationary operands are pre-interleaved for SwInterleave."""
    zqT8 = _to_fp8(Z_q).T                            # [512, 256]
    zqT = zqT8.reshape(KSUB, 128, B).transpose(1, 0, 2)  # [128, KSUB, B]
    # part-2 stationary: [128, kp*2+bt, 256] interleaved
    zz = zqT8.reshape(KPAIR, 2, 128, 2, 128)         # [kp, i, p, bt, col]
    zqTi = (
        _interleave(zz[:, 0], zz[:, 1])
        .transpose(1, 0, 2, 3)
        .reshape(128, KSUB, 256)
    )
    zq = np.ascontiguousarray(np.concatenate([zqT, zqTi], axis=1))  # [128, 8, 256]

    qT = np.ascontiguousarray(_to_fp8(queue).T)      # [512, 65536]
    cT = np.ascontiguousarray(_to_fp8(centroids).T)  # [512, 50000]

    in_maps = []
    for i in range(NCORES):
        q_sh = qT[:, i * QSH : (i + 1) * QSH]        # [512, 8192]
        # [kp, i, p, h, jl, col]
        qq = q_sh.reshape(KPAIR, 2, 128, QCHUNK, JSUB, 128)
        q_sh = np.ascontiguousarray(
            _interleave(qq[:, 0], qq[:, 1]).transpose(2, 1, 3, 0, 4)
        )  # [QCHUNK, 128, JSUB, KPAIR, 256]
        c_sh = np.zeros((C, CSH_PAD), ml_dtypes.float8_e4m3fn)
        c_sh[:, :CSH] = cT[:, i * CSH : (i + 1) * CSH]
        # cTb = first 2 matmul chunks (small, lands first); cTa = the rest
        c_b = np.ascontiguousarray(
            c_sh[:, : 2 * CW].reshape(KSUB, 128, 2 * CW).transpose(1, 0, 2)
        )  # [128, KSUB, 2*CW]
        c_a = np.ascontiguousarray(
            c_sh[:, 2 * CW :].reshape(KSUB, 128, 3, 4 * CW).transpose(2, 1, 0, 3)
        )  # [3, 128, KSUB, 4*CW]
        in_maps.append({"zq": zq, "qT": q_sh, "cTa": c_a, "cTb": c_b})
    return in_maps


def kernel(Z_q, Z_k, queue, centroids, kmeans_temp, neg_raw):
    global last_exec_time_ns
    from concourse.bass_utils import run_bass_kernel_spmd

    nc = _get_nc()
    in_maps = _prep_inputs(Z_q, queue, centroids)

    trace = bool(int(os.environ.get("MOCO_BASS_TRACE", "0")))
    out = run_bass_kernel_spmd(nc, in_maps, core_ids=list(range(NCORES)), trace=trace)
    last_exec_time_ns = out.exec_time_ns
    res = out.results

    # ---- host combine (tiny) ----
    lp = (Z_q.astype(np.float64) * Z_k.astype(np.float64)).sum(axis=1)  # l_pos
    lp_t = lp / INFO_TEMP

    # part-1 loss: logsumexp over [l_pos | l_neg]/T per batch row.
    # Device partials are unshifted sums of e^{s/T} (|s/T| <= ~4).
    S = np.zeros(B, np.float64)
    for r in res:
        S += r["fin"][0].astype(np.float64)
    S += np.exp(lp_t)
    lse1 = np.log(S)
    loss1 = np.mean(lse1 - lp_t)

    # accuracy: exact despite fp8 scores.  The device reduces the fp8 exp
    # tiles over the batch axis (rm = max_b exp(s/T), fp32-exact); every
    # row with margin < MARGIN is re-checked on the host in full precision.
    rm_full = np.empty(QUEUE, np.float64)
    for i, r in enumerate(res):
        rm = r["rm"].astype(np.float64).transpose(1, 2, 0).reshape(-1)
        rm_full[i * QSH : (i + 1) * QSH] = np.log(rm) * INFO_TEMP

    # s0 computed exactly on the host (33 MFLOP) -- only rm comes from
    # the device, so the margin test has one noisy side instead of two
    s0_full = queue.astype(np.float64) @ Z_q[0].astype(np.float64)
    cand = (rm_full - s0_full) < MARGIN
    cols = np.nonzero(cand)[0]
    sub = Z_q.astype(np.float64) @ queue[cols].astype(np.float64).T  # [B, ncand]
    count = float((sub[0] >= sub.max(axis=0)).sum())
    count += float(lp[0] >= lp.max())
    accuracy = count / (1 + QUEUE)

    # part-2: global argmax over centroids (== argmin of ||c||^2 - 2 s).
    # s2 arrives in fp8; the argmax (and the positive logit) is resolved
    # exactly by re-checking every near-max column in full precision.
    s2_full = np.empty((B, NCL), np.float32)
    for i, r in enumerate(res):
        sh = r["s2"].astype(np.float32).transpose(2, 0, 1, 3).reshape(B, CSH_PAD)
        s2_full[:, i * CSH : (i + 1) * CSH] = sh[:, :CSH]
    s2_full /= DOT_SCALE

    kt = kmeans_temp.astype(np.float64)
    Zq64 = Z_q.astype(np.float64)
    ce64 = centroids.astype(np.float64)
    mx = s2_full.max(axis=1)
    I = np.empty(B, np.int64)
    pl_pos = np.empty(B)
    for b in range(B):
        cnd = np.nonzero(s2_full[b] >= mx[b] - MARGIN2)[0]
        ex = ce64[cnd] @ Zq64[b]
        k = int(np.argmax(ex))
        I[b] = cnd[k]
        pl_pos[b] = ex[k] / kt[cnd[k]]

    neg_idx = neg_raw + (neg_raw >= I[:, None]).astype(neg_raw.dtype)
    pl_neg = (
        np.take_along_axis(s2_full, neg_idx, axis=1).astype(np.float64)
        / kt[neg_idx]
    )
    plogits = np.concatenate([pl_pos[:, None], pl_neg], axis=1)
    m = plogits.max(axis=1)
    plse = np.log(np.exp(plogits - m[:, None]).sum(axis=1)) + m
    ploss = np.mean(plse - pl_pos)

    loss = loss1 + PROTO_FACTOR * ploss
    return np.float32(loss), np.float32(accuracy)


# revision 7
# speedup vs baseline: 1.0529x; 1.0529x over previous
"""MoCo loss (InfoNCE over a 65536-entry queue + proto-NCE over 50000
k-means centroids) on 8 Trainium2 NeuronCores.

fp8e4m3 operands with DoubleRowSwInterleave matmuls (2 contraction
subtiles per instruction; the stationary operand is pre-interleaved on
the host -- A/B k-layer pairs per column, columns reversed -- so the
weight load reads contiguously).  Tables are scaled by 16 per operand
(dots come out x256) and quantized to e4m3 on the host.

Per core (tables sharded by row, Z_q replicated):

  part 2 (centroid shard, zq stationary / centroids moving):
    - s2 = Z_q @ c_shard.T  (PE, fp32 acc), cast fp8 (DVE/ACT), export
      (argmax + exclusion gather on the host)
  part 1 (queue shard, queue stationary / zq moving):
    - s1 = q_shard @ Z_q.T                 (PE)
    - exp(s1/T) in fp8  (no shift; values in [e^-4, e^4])   (ACT)
    - per-queue-row max of the exp tiles -> rm export       (DVE)
    - ones DoubleRow matmul partition sum -> per-batch partial
      sum-of-exp                                            (PE)

The kernel streams ~7.3 MiB of table reads per core on both HWDGE
rings (sync + scalar); part-1 tiles and part-2 chunks are emitted
interleaved in DMA-arrival order so the PE never starves; part-2's
last chunks are the final PE work so the end-of-kernel chain is just
cast -> one merged export (s2 tail + rm + fin share one DRAM tensor
with fat 6.5 KiB lines).

The host combines per-core partials (logsumexp merge, global argmax,
exclusion gather + 513-wide softmax).  Both outputs stay exact despite
fp8 noise: the host computes s0 = queue @ Z_q[0] itself and re-checks
every queue row within MARGIN of the device rowmax -- and every
centroid column within MARGIN2 of its row max -- with full-precision
dot products (fp8 dot error is < 0.01).
"""

import os
import numpy as np
import ml_dtypes

B, C = 256, 512
QUEUE, NCL, NNEG = 65536, 50000, 512
INFO_TEMP = 0.07
PROTO_FACTOR = 0.5
NCORES = 8
QSH = QUEUE // NCORES          # 8192 queue rows per core
CSH = NCL // NCORES            # 6250 centroid rows per core
CSH_PAD = 6272                 # 14 * 448
CCH = 14                       # s2 matmul chunks
CW = CSH_PAD // CCH            # 448
KSUB = C // 128                # 4 contraction subtiles
KPAIR = KSUB // 2              # 2 DoubleRow pairs
NBT = 16                       # part-1 big tiles (512 queue rows each)
QCHUNK = 4                     # qT DMA chunks (8 KiB per-partition lines)
JW = QSH // QCHUNK             # 2048
JSUB = JW // 128               # 16 queue subtiles per chunk
FP8_SCALE = 16.0               # per-operand scale; dots come out x256
DOT_SCALE = FP8_SCALE * FP8_SCALE
MARGIN = 0.05                  # host re-check threshold (unscaled units)
MARGIN2 = 0.045                # part-2 argmax re-check threshold

# merged export regions (bytes per partition inside "out")
S2_BYTES = CCH * 2 * CW        # 12544
RM_OFF = S2_BYTES              # [128, NBT*4] fp32 = 256 B
FIN_OFF = RM_OFF + NBT * 4 * 4  # [128, 256] fp32 = 1024 B
OUT_BYTES = FIN_OFF + B * 4    # 13824
S2A_BYTES = 8 * 2 * CW         # early export: chunks 0-7 (7168 B lines)

DVE_CAST_CH = frozenset(range(8))   # s2 casts on DVE (early chunks); rest ACT
# rm reduce groups: big groups early, small at the tail so the last
# reduce has a short critical chain after the last exp tile
RM_GROUPS = ((0, 4), (4, 8), (8, 12), (12, 14), (14, 15), (15, 16))

_CACHE = {}

# exec time of the last device run (ns), populated when tracing is on
last_exec_time_ns = None


def _build():
    import concourse.bass as bass
    import concourse.tile as tile
    from concourse import bacc, mybir

    dt = mybir.dt
    DRI = mybir.MatmulPerfMode.DoubleRowSwInterleave
    nc = bacc.Bacc(
        "TRN2", target_bir_lowering=False, debug=False, num_devices=NCORES
    )

    # ---- DRAM I/O (all partition-major so every DMA is [128, N] flat) ----
    zq_d = nc.dram_tensor("zq", [128, 8, 256], dt.float8e4, kind="ExternalInput").ap()
    qT_d = nc.dram_tensor(
        "qT", [QCHUNK, 128, JSUB, KPAIR, 256], dt.float8e4, kind="ExternalInput"
    ).ap()
    cTa_d = nc.dram_tensor(
        "cTa", [3, 128, KSUB, 4 * CW], dt.float8e4, kind="ExternalInput"
    ).ap()
    cTb0_d = nc.dram_tensor(
        "cTb0", [128, KSUB, CW], dt.float8e4, kind="ExternalInput"
    ).ap()
    cTb1_d = nc.dram_tensor(
        "cTb1", [128, KSUB, CW], dt.float8e4, kind="ExternalInput"
    ).ap()

    # merged output: s2 fp8 [128, CCH, 2, CW] | rm fp32 [128, NBT, 4] |
    # fin fp32 [128, 256] -- exported as two flat fp8 DMAs with fat lines
    out_d = nc.dram_tensor(
        "out", [128, OUT_BYTES], dt.float8e4, kind="ExternalOutput"
    ).ap()

    with tile.TileContext(nc) as tc:
        with (
            tc.tile_pool(name="const", bufs=1) as cpool,
            tc.tile_pool(name="ps1", bufs=2, space="PSUM") as ps1,
            tc.tile_pool(name="psum1s", bufs=1, space="PSUM") as ps1s,
            tc.tile_pool(name="ps2", bufs=3, space="PSUM") as ps2,
        ):
            # ---- resident SBUF tensors ----
            zq_sb = cpool.tile([128, 8, 256], dt.float8e4)
            cT_sb = [
                cpool.tile(
                    [128, KSUB, 4 * CW], dt.float8e4, name=f"cTa{ch}", tag=f"cT{ch}"
                )
                for ch in range(3)
            ]
            cTb_sb = [
                cpool.tile([128, KSUB, CW], dt.float8e4, name=f"cTb{i}", tag=f"cTb{i}")
                for i in range(2)
            ]
            qt_sb = [
                cpool.tile(
                    [128, JSUB, KPAIR, 256], dt.float8e4, name=f"qt{h}", tag=f"qt{h}"
                )
                for h in range(QCHUNK)
            ]

            # Input DMAs on the two HWDGE rings (sync + scalar); issue order
            # = per-ring arrival order, interleaved to match the PE emission
            # order below.  cTa2 (the last part-2 chunks) lands last so the
            # end-of-kernel chain is short.
            nc.sync.dma_start(zq_sb[:], zq_d[:])
            nc.scalar.dma_start(cTb_sb[0][:], cTb0_d[:])
            nc.scalar.dma_start(cTb_sb[1][:], cTb1_d[:])
            nc.sync.dma_start(qt_sb[0][:], qT_d[0])
            nc.scalar.dma_start(cT_sb[0][:], cTa_d[0])
            nc.sync.dma_start(cT_sb[1][:], cTa_d[1])
            nc.scalar.dma_start(qt_sb[1][:], qT_d[1])
            nc.sync.dma_start(qt_sb[2][:], qT_d[2])
            nc.scalar.dma_start(qt_sb[3][:], qT_d[3])
            nc.scalar.dma_start(cT_sb[2][:], cTa_d[2])

            # interleave/column-reversal of all-ones is all-ones
            ones_sb = cpool.tile([128, 256], dt.float8e4)
            nc.vector.memset(ones_sb[:], 1.0)

            # merged export tile + typed views
            out_sb = cpool.tile([128, OUT_BYTES], dt.float8e4)
            s2_v = out_sb[:, 0:S2_BYTES].rearrange(
                "p (c b w) -> p c b w", c=CCH, b=2, w=CW
            )
            rm_v = out_sb[:, RM_OFF:FIN_OFF].bitcast(dt.float32)   # [128, 64]
            fin_v = out_sb[:, FIN_OFF:OUT_BYTES].bitcast(dt.float32)  # [128, 256]

            exp_all = cpool.tile([128, NBT, 4, B], dt.float8e4)
            exp_tiles = [exp_all[:, t] for t in range(NBT)]

            fin_sb = None  # ACT writes fin via fin_v

            # ---- emission helpers ----
            def emit_chunk(ch):
                """part-2 chunk: s2[:, ch] = Z_q @ c_chunk.T, cast to fp8."""
                if ch < 2:
                    cmov, w = cTb_sb[ch], 0
                else:
                    dch, w = divmod(ch - 2, 4)
                    cmov = cT_sb[dch]
                for bt in range(2):
                    s2_ps = ps2.tile([128, CW], dt.float32, tag="s2")
                    for kp in range(KPAIR):
                        nc.tensor.matmul(
                            s2_ps[:],
                            zq_sb[:, 4 + 2 * kp + bt, :],
                            cmov[:, 2 * kp : 2 * kp + 2, w * CW : (w + 1) * CW],
                            start=(kp == 0),
                            stop=(kp == KPAIR - 1),
                            perf_mode=DRI,
                        )
                    if ch in DVE_CAST_CH:
                        nc.vector.tensor_copy(s2_v[:, ch, bt, :], s2_ps[:])
                    else:
                        nc.scalar.copy(s2_v[:, ch, bt, :], s2_ps[:])

            rm_after = {b - 1: (a, b) for a, b in RM_GROUPS}

            def emit_tile(t, p1s_ps):
                """part-1 tile: 512 queue rows -> exp fp8; lagged ones-sum;
                rm reduce at group boundaries."""
                s1_ps = ps1.tile([128, 4, B], dt.float32, tag="s1")
                for q in range(4):
                    jt = t * 4 + q
                    h, jl = divmod(jt, JSUB)
                    for kp in range(KPAIR):
                        nc.tensor.matmul(
                            s1_ps[:, q, :],
                            qt_sb[h][:, jl, kp, :],
                            zq_sb[:, 2 * kp : 2 * kp + 2, :],
                            start=(kp == 0),
                            stop=(kp == KPAIR - 1),
                            perf_mode=DRI,
                        )
                nc.scalar.activation(
                    exp_tiles[t][:],
                    s1_ps[:],
                    mybir.ActivationFunctionType.Exp,
                    scale=1.0 / (DOT_SCALE * INFO_TEMP),
                )
                if t > 1:
                    # two tiles behind: ACT's exp has a full tile of slack
                    for g in range(2):
                        nc.tensor.matmul(
                            p1s_ps[:],
                            ones_sb[:],
                            exp_tiles[t - 2][:, 2 * g : 2 * g + 2, :],
                            start=(t == 2 and g == 0),
                            stop=False,
                            perf_mode=DRI,
                        )
                if t in rm_after:
                    a, b = rm_after[t]
                    nc.vector.tensor_reduce(
                        rm_v[:, a * 4 : b * 4],
                        exp_all[:, a:b],
                        axis=mybir.AxisListType.X,
                        op=mybir.AluOpType.max,
                    )

            # ---- interleaved emission (matches DMA arrival order) ----
            p1s_ps = ps1s.tile([128, B], dt.float32)   # sum-of-exp accum

            emit_chunk(0)
            emit_chunk(1)
            seq = [("t", 0), ("c", 2), ("t", 1), ("c", 3), ("t", 2), ("c", 4),
                   ("t", 3), ("c", 5), ("t", 4), ("c", 6), ("t", 5), ("c", 7)]
            for kind, i in seq:
                if kind == "t":
                    emit_tile(i, p1s_ps)
                else:
                    emit_chunk(i)
            # chunks 0-7 cast: early export (overlaps the rest of the kernel)
            nc.gpsimd.dma_start(out_d[:, 0:S2A_BYTES], out_sb[:, 0:S2A_BYTES])
            for kind, i in [("t", 6), ("c", 8), ("t", 7), ("c", 9)]:
                if kind == "t":
                    emit_tile(i, p1s_ps)
                else:
                    emit_chunk(i)
            for t in range(8, NBT):
                emit_tile(t, p1s_ps)
            # flush the lagged ones-sum (tiles 14, 15)
            for t in (NBT - 2, NBT - 1):
                for g in range(2):
                    nc.tensor.matmul(
                        p1s_ps[:],
                        ones_sb[:],
                        exp_tiles[t][:, 2 * g : 2 * g + 2, :],
                        start=False,
                        stop=(t == NBT - 1 and g == 1),
                        perf_mode=DRI,
                    )
            nc.scalar.copy(fin_v[:], p1s_ps[:])
            # last part-2 chunks: the final PE work (short export chain)
            for ch in range(10, CCH):
                emit_chunk(ch)
            # merged final export: s2 chunks 8-13 + rm + fin (6656 B lines)
            nc.sync.dma_start(
                out_d[:, S2A_BYTES:OUT_BYTES], out_sb[:, S2A_BYTES:OUT_BYTES]
            )

    nc.compile()
    return nc


def _get_nc():
    if "nc" not in _CACHE:
        _CACHE["nc"] = _build()
    return _CACHE["nc"]


def _to_fp8(x):
    return (x * FP8_SCALE).astype(ml_dtypes.float8_e4m3fn)


def _interleave(A, B):
    """SwInterleave weight layout: mem[p, 2*jj+i] = layer_i[p, 127-jj].
    A, B: [..., 128, 128] (partition, column)."""
    return np.stack([A[..., ::-1], B[..., ::-1]], axis=-1).reshape(
        *A.shape[:-1], 256
    )


def _prep_inputs(Z_q, queue, centroids):
    """Host-side shard prep: x16 scale + e4m3 quantization + transpose to
    [C, rows], then partition-major chunk layouts so each DMA is a flat
    [128, N].  Stationary operands are pre-interleaved for SwInterleave."""
    zqT8 = _to_fp8(Z_q).T                            # [512, 256]
    zqT = zqT8.reshape(KSUB, 128, B).transpose(1, 0, 2)  # [128, KSUB, B]
    # part-2 stationary: [128, kp*2+bt, 256] interleaved
    zz = zqT8.reshape(KPAIR, 2, 128, 2, 128)         # [kp, i, p, bt, col]
    zqTi = (
        _interleave(zz[:, 0], zz[:, 1])
        .transpose(1, 0, 2, 3)
        .reshape(128, KSUB, 256)
    )
    zq = np.ascontiguousarray(np.concatenate([zqT, zqTi], axis=1))  # [128, 8, 256]

    qT = np.ascontiguousarray(_to_fp8(queue).T)      # [512, 65536]
    cT = np.ascontiguousarray(_to_fp8(centroids).T)  # [512, 50000]

    in_maps = []
    for i in range(NCORES):
        q_sh = qT[:, i * QSH : (i + 1) * QSH]        # [512, 8192]
        # [kp, i, p, h, jl, col]
        qq = q_sh.reshape(KPAIR, 2, 128, QCHUNK, JSUB, 128)
        q_sh = np.ascontiguousarray(
            _interleave(qq[:, 0], qq[:, 1]).transpose(2, 1, 3, 0, 4)
        )  # [QCHUNK, 128, JSUB, KPAIR, 256]
        c_sh = np.zeros((C, CSH_PAD), ml_dtypes.float8_e4m3fn)
        c_sh[:, :CSH] = cT[:, i * CSH : (i + 1) * CSH]
        # cTb0/cTb1 = first 2 matmul chunks (small, land first); cTa = rest
        c_b0 = np.ascontiguousarray(
            c_sh[:, :CW].reshape(KSUB, 128, CW).transpose(1, 0, 2)
        )
        c_b1 = np.ascontiguousarray(
            c_sh[:, CW : 2 * CW].reshape(KSUB, 128, CW).transpose(1, 0, 2)
        )
        c_a = np.ascontiguousarray(
            c_sh[:, 2 * CW :].reshape(KSUB, 128, 3, 4 * CW).transpose(2, 1, 0, 3)
        )  # [3, 128, KSUB, 4*CW]
        in_maps.append({"zq": zq, "qT": q_sh, "cTa": c_a,
                        "cTb0": c_b0, "cTb1": c_b1})
    return in_maps


def kernel(Z_q, Z_k, queue, centroids, kmeans_temp, neg_raw):
    global last_exec_time_ns
    from concourse.bass_utils import run_bass_kernel_spmd

    nc = _get_nc()
    in_maps = _prep_inputs(Z_q, queue, centroids)

    trace = bool(int(os.environ.get("MOCO_BASS_TRACE", "0")))
    out = run_bass_kernel_spmd(nc, in_maps, core_ids=list(range(NCORES)), trace=trace)
    last_exec_time_ns = out.exec_time_ns
    res = out.results

    # decode the merged export per core
    def regions(r):
        raw = np.ascontiguousarray(r["out"])         # [128, OUT_BYTES] fp8
        s2 = raw[:, :S2_BYTES].astype(np.float32).reshape(128, CCH, 2, CW)
        rm = np.ascontiguousarray(raw[:, RM_OFF:FIN_OFF]).view(np.float32)
        fin = np.ascontiguousarray(raw[:, FIN_OFF:]).view(np.float32)
        return s2, rm, fin

    decoded = [regions(r) for r in res]

    # ---- host combine (tiny) ----
    lp = (Z_q.astype(np.float64) * Z_k.astype(np.float64)).sum(axis=1)  # l_pos
    lp_t = lp / INFO_TEMP

    # part-1 loss: logsumexp over [l_pos | l_neg]/T per batch row.
    # Device partials are unshifted sums of e^{s/T} (|s/T| <= ~4).
    S = np.zeros(B, np.float64)
    for _, _, fin in decoded:
        S += fin[0].astype(np.float64)
    S += np.exp(lp_t)
    lse1 = np.log(S)
    loss1 = np.mean(lse1 - lp_t)

    # accuracy: exact despite fp8 scores.  The device reduces the fp8 exp
    # tiles over the batch axis (rm = max_b exp(s/T), fp32-exact); every
    # row with margin < MARGIN is re-checked on the host in full precision.
    rm_full = np.empty(QUEUE, np.float64)
    for i, (_, rm, _) in enumerate(decoded):
        # rm[p, jt] -> queue row j = jt*128 + p
        rm_full[i * QSH : (i + 1) * QSH] = (
            np.log(rm.astype(np.float64).T.reshape(-1)) * INFO_TEMP
        )

    # s0 computed exactly on the host (33 MFLOP) -- only rm comes from
    # the device, so the margin test has one noisy side instead of two
    s0_full = queue.astype(np.float64) @ Z_q[0].astype(np.float64)
    cand = (rm_full - s0_full) < MARGIN
    cols = np.nonzero(cand)[0]
    sub = Z_q.astype(np.float64) @ queue[cols].astype(np.float64).T  # [B, ncand]
    count = float((sub[0] >= sub.max(axis=0)).sum())
    count += float(lp[0] >= lp.max())
    accuracy = count / (1 + QUEUE)

    # part-2: global argmax over centroids (== argmin of ||c||^2 - 2 s).
    # s2 arrives in fp8; the argmax (and the positive logit) is resolved
    # exactly by re-checking every near-max column in full precision.
    s2_full = np.empty((B, NCL), np.float32)
    for i, (s2, _, _) in enumerate(decoded):
        sh = s2.transpose(2, 0, 1, 3).reshape(B, CSH_PAD)
        s2_full[:, i * CSH : (i + 1) * CSH] = sh[:, :CSH]
    s2_full /= DOT_SCALE

    kt = kmeans_temp.astype(np.float64)
    Zq64 = Z_q.astype(np.float64)
    ce64 = centroids.astype(np.float64)
    mx = s2_full.max(axis=1)
    I = np.empty(B, np.int64)
    pl_pos = np.empty(B)
    for b in range(B):
        cnd = np.nonzero(s2_full[b] >= mx[b] - MARGIN2)[0]
        ex = ce64[cnd] @ Zq64[b]
        k = int(np.argmax(ex))
        I[b] = cnd[k]
        pl_pos[b] = ex[k] / kt[cnd[k]]

    neg_idx = neg_raw + (neg_raw >= I[:, None]).astype(neg_raw.dtype)
    pl_neg = (
        np.take_along_axis(s2_full, neg_idx, axis=1).astype(np.float64)
        / kt[neg_idx]
    )
    plogits = np.concatenate([pl_pos[:, None], pl_neg], axis=1)
    m = plogits.max(axis=1)
    plse = np.log(np.exp(plogits - m[:, None]).sum(axis=1)) + m
    ploss = np.mean(plse - pl_pos)

    loss = loss1 + PROTO_FACTOR * ploss
    return np.float32(loss), np.float32(accuracy)
